# revision 34
# baseline (speedup 1.0000x reference)
"""MoE gated 3x3 conv (eval path) on 8 trn2 NeuronCores.

Strategy:
- Routing (tiny: [16,64]@[64,16] -> softmax -> top-4 gates) and the scalar
  aux loss are computed on host; the gates determine a per-sample merged
  conv weight  Wc[b] = sum_e gates[b,e] * W[e]  (conv is linear in the
  weights, and zero-gate experts contribute nothing), so the device does
  exactly one 3x3 conv per sample instead of num_experts of them.
- Data parallel over batch: 2 samples per core. Sample 0 lives on SBUF
  partitions 0-63, sample 1 on partitions 64-127, so the two per-sample
  matmul chains are row-tiled (tile_position (0,0)/(64,0)) and the PE
  runs both concurrently.
- Width-65 padded image layout: row i of the padded image is
  [0, x[i-1, 0..63]], with zero rows above and below. A single zero
  column between consecutive rows serves as BOTH the right pad of row i
  and the left pad of row i+1, so every conv tap (dy, dx) is a pure flat
  shift by dy*65+dx and the matmul moving operand is fully contiguous.
- The conv is 9 shift-matmuls accumulating in PSUM over flat 512-wide
  windows (not row-aligned; junk columns are stripped on host). 9 windows
  cover the 64x65 flat output space.
- Default MODE "bf16x4": bf16 matmuls with window PAIRS on the 2x2 PE
  tile_position grid -- 4 accumulation chains (2 samples x 2 windows) run
  concurrently on the 128x128 array; fp32 PSUM accumulation; ~2.2e-3
  scale-relative output error. MODE "f32r2" is a float32r (TF32-like)
  fallback at ~1.4e-4 error, ~35% slower.
- Per-window bias-adds are split across the Vector and Scalar engines;
  x is loaded in 5 range-pieces so early windows' matmuls start while the
  rest of the image is still in flight.
"""

import ml_dtypes
import numpy as np

import concourse.bacc as bacc
import concourse.tile as tile
from concourse import mybir
from concourse.bass_utils import run_bass_kernel_spmd

N_CORES = 8
B, CIN, COUT, E = 16, 64, 64, 16
H = W_SP = 64
KTOP = 4
PW = W_SP + 1          # 65: one shared zero column per row
NROW = 73              # padded rows (top zero, data, bottom zero + overrun)
XFLAT = NROW * PW      # 4745
NMM = 512              # flat window width per PSUM accumulation group
NWIN = 9               # ceil(64*65 / 512)
XPIECES = (648, 1672, 2696, 3720, XFLAT)  # x-load split points (flat, excl.)
F32 = mybir.dt.float32

# "bf16x4": bf16 matmuls, 2 samples x 2 windows concurrent on the 2x2 PE
#           tile grid (~2.5e-3 scale-relative output error).
# "f32r2":  float32r (TF32-like) matmuls, 2 samples concurrent via row
#           tiling only (~1.4e-4 error; fp32r PSUM must start at
#           partition 0, so no column tiling).
MODE = "bf16x4"

_PROGRAM_CACHE = {}


def _routing_gates(x, w_gate):
    """Eval-path gates, mirroring the reference: softmax over clean logits,
    top-4 renormalized. [B, E] float32."""
    gate_x = x.reshape(B, CIN, H * W_SP).mean(axis=2)      # [B, Cin]
    logits = gate_x.astype(np.float32) @ w_gate            # [B, E]
    m = logits.max(axis=1, keepdims=True)
    ex = np.exp(logits - m)
    sm = ex / ex.sum(axis=1, keepdims=True)
    idx = np.argsort(-sm, axis=1, kind="stable")[:, :KTOP]
    vals = np.take_along_axis(sm, idx, axis=1)
    gk = vals / (vals.sum(axis=1, keepdims=True) + 1e-6)
    gates = np.zeros((B, E), np.float32)
    np.put_along_axis(gates, idx, gk.astype(np.float32), axis=1)
    return gates


def _aux_loss(gates):
    load = (gates > 0).sum(axis=0).astype(np.float32)
    importance = gates.sum(axis=0).astype(np.float32)

    def cv_sq(v):
        return v.var(ddof=1) / (v.mean() ** 2 + 1e-10)

    return np.float32((cv_sq(importance) + cv_sq(load)) * 0.01)


def _build_program():
    mm_dt = mybir.dt.bfloat16 if MODE == "bf16x4" else mybir.dt.float32r
    nc = bacc.Bacc("TRN2", target_bir_lowering=False, debug=False,
                   num_devices=N_CORES)
    # xw = [wct | padded x] so one DMA delivers the weights plus the first
    # windows' image data
    xw = nc.dram_tensor("xw", [128, 9 * COUT + XFLAT], mm_dt,
                        kind="ExternalInput").ap()
    bias2 = nc.dram_tensor("bias2", [128, 2], F32, kind="ExternalInput").ap()
    y2p = nc.dram_tensor("y2p", [COUT, NWIN, 2 * NMM], F32,
                         kind="ExternalOutput").ap()
    y2v = y2p  # [co, window, sample*NMM] -- window slice is contiguous 2D
    W0 = 9 * COUT

    with tile.TileContext(nc) as tc:
        with tc.tile_pool(name="xs", bufs=1) as xpool, \
             tc.tile_pool(name="w", bufs=1) as wpool, \
             tc.tile_pool(name="out", bufs=3) as opool, \
             tc.tile_pool(name="ps", bufs=3, space="PSUM") as pspool:
            xsw = xpool.tile([128, W0 + XFLAT], mm_dt)
            xs = xsw[:, W0:]
            w3 = xsw[:, 0:W0].rearrange("p (t c) -> p t c", t=9)
            bsb = wpool.tile([128, 2], F32)
            nc.sync.dma_start(out=xsw[:, 0:W0 + XPIECES[0]],
                              in_=xw[:, 0:W0 + XPIECES[0]])
            nc.scalar.dma_start(out=bsb, in_=bias2)
            lo = XPIECES[0]
            for hi in XPIECES[1:]:
                nc.sync.dma_start(out=xsw[:, W0 + lo:W0 + hi],
                                  in_=xw[:, W0 + lo:W0 + hi])
                lo = hi

            if MODE == "bf16x4":
                _body_bf16x4(nc, pspool, opool, xs, w3, bsb, y2v)
            else:
                _body_f32r2(nc, pspool, opool, xs, w3, bsb, y2v)
    nc.compile()
    return nc


def _body_f32r2(nc, pspool, opool, xs, w3, bsb, y2v):
    for g in range(NWIN):
        psA = pspool.tile([COUT, NMM], F32, tag="psA")
        psB = pspool.tile([COUT, NMM], F32, tag="psB")
        for t in range(9):
            dy, dx = divmod(t, 3)
            o = g * NMM + dy * PW + dx
            nc.tensor.matmul(psA, lhsT=w3[0:64, t, :],
                             rhs=xs[0:64, o:o + NMM],
                             start=(t == 0), stop=(t == 8))
            nc.tensor.matmul(psB, lhsT=w3[64:128, t, :],
                             rhs=xs[64:128, o:o + NMM],
                             start=(t == 0), stop=(t == 8))
        oAB = opool.tile([COUT, 2 * NMM], F32, tag="oAB")
        nc.vector.tensor_scalar_add(oAB[:, 0:NMM], psA, bsb[0:64, 0:1])
        nc.vector.tensor_scalar_add(oAB[:, NMM:], psB, bsb[0:64, 1:2])
        nc.sync.dma_start(out=y2v[:, g, :], in_=oAB)


def _body_bf16x4(nc, pspool, opool, xs, w3, bsb, y2v):
    # windows in pairs: chains (sample, window) on PE tile grid
    # (0,g)=(row0,col0) (1,g)=(row64,col0) (0,g1)=(row0,col64)
    # (1,g1)=(row64,col64); t1 holds sample-0 window g (top) and g1
    # (bottom), t2 the same for sample 1.
    # The last (odd) window only has 64 valid flat columns, so it runs
    # with a narrow moving operand and a small output DMA.
    for g in (*range(0, NWIN - 1, 2), NWIN - 1):
        g1 = g + 1
        pair = g1 < NWIN
        n = NMM if pair else 128
        nv = NMM if pair else 64   # valid/stored columns
        t1 = pspool.tile([128, NMM], F32, tag="t1")
        t2 = pspool.tile([128, NMM], F32, tag="t2")
        for t in range(9):
            dy, dx = divmod(t, 3)
            o = g * NMM + dy * PW + dx
            nc.tensor.matmul(t1[0:64, 0:n], lhsT=w3[0:64, t, :],
                             rhs=xs[0:64, o:o + n],
                             start=(t == 0), stop=(t == 8))
            nc.tensor.matmul(t2[0:64, 0:n], lhsT=w3[64:128, t, :],
                             rhs=xs[64:128, o:o + n],
                             start=(t == 0), stop=(t == 8))
            if pair:
                o1 = o + NMM
                nc.tensor.matmul(t1[64:128], lhsT=w3[0:64, t, :],
                                 rhs=xs[0:64, o1:o1 + NMM],
                                 start=(t == 0), stop=(t == 8))
                nc.tensor.matmul(t2[64:128], lhsT=w3[64:128, t, :],
                                 rhs=xs[64:128, o1:o1 + NMM],
                                 start=(t == 0), stop=(t == 8))
        oT = opool.tile([128, 2 * NMM], F32, tag="oT")
        nc.vector.tensor_scalar_add(oT[0:64, 0:nv], t1[0:64, 0:nv],
                                    bsb[0:64, 0:1])
        nc.scalar.add(oT[0:64, NMM:NMM + nv], t2[0:64, 0:nv], bsb[0:64, 1:2])
        if pair:
            nc.sync.dma_start(out=y2v[:, g, :], in_=oT[0:64])
        else:
            nc.sync.dma_start(
                out=y2v[:, g, :].rearrange("p (s n) -> p s n", s=2)[:, :, 0:nv],
                in_=oT[0:64].rearrange("p (s n) -> p s n", s=2)[:, :, 0:nv])
        if pair:
            nc.vector.tensor_scalar_add(oT[64:128, 0:NMM], t1[64:128],
                                        bsb[64:128, 0:1])
            nc.scalar.add(oT[64:128, NMM:], t2[64:128], bsb[64:128, 1:2])
            nc.sync.dma_start(out=y2v[:, g1, :], in_=oT[64:128])


def get_program():
    if "nc" not in _PROGRAM_CACHE:
        _PROGRAM_CACHE["nc"] = _build_program()
    return _PROGRAM_CACHE["nc"]


def _pad_x(xpair):
    """[2, CIN, H, W] -> [128, XFLAT] width-65 padded flat layout."""
    out = np.zeros((2, CIN, NROW, PW), np.float32)
    out[:, :, 1:H + 1, 1:] = xpair
    return out.reshape(2 * CIN, XFLAT)


def make_in_maps(x, Wc, bc):
    """Per-core input maps: 2 samples per core."""
    in_maps = []
    for c in range(N_CORES):
        s0, s1 = 2 * c, 2 * c + 1
        # wct[ci + 64*s, t*64 + co] = Wc[sample, co, ci, t]
        w0 = Wc[s0].reshape(COUT, CIN, 9).transpose(1, 2, 0)
        w1 = Wc[s1].reshape(COUT, CIN, 9).transpose(1, 2, 0)
        wctm = np.concatenate([w0, w1], axis=0).reshape(128, 9 * COUT)
        bias2 = np.stack([bc[s0], bc[s1]], axis=1)      # [COUT, 2]
        bias2 = np.concatenate([bias2, bias2], axis=0)  # both psum halves
        xwm = np.concatenate([wctm, _pad_x(x[s0:s1 + 1])], axis=1)
        if MODE == "bf16x4":
            xwm = xwm.astype(ml_dtypes.bfloat16)
        in_maps.append({
            "xw": xwm,
            "bias2": np.ascontiguousarray(bias2),
        })
    return in_maps


# compaction index: y[.., h, w] = y2p[.., GIDX[h, w], JIDX[h, w]]
_f = np.arange(H)[:, None] * PW + np.arange(W_SP)[None, :]
GIDX = _f // NMM
JIDX = _f % NMM


def gather_y(results):
    y = np.empty((B, COUT, H, W_SP), np.float32)
    for c in range(N_CORES):
        yp = results[c]["y2p"].reshape(COUT, NWIN, 2, NMM)
        for s in range(2):
            y[2 * c + s] = yp[:, GIDX, s, JIDX]
    return y


def kernel(**inputs):
    x = np.asarray(inputs["x"], dtype=np.float32)
    w_gate = np.asarray(inputs["w_gate"], dtype=np.float32)
    W = np.asarray(inputs["W"], dtype=np.float32)
    b = np.asarray(inputs["b"], dtype=np.float32)
    # train is eval-only in the reference; the noise branch never runs.

    gates = _routing_gates(x, w_gate)
    loss = _aux_loss(gates)
    Wc = np.tensordot(gates, W.reshape(E, -1), axes=(1, 0)) \
        .reshape(B, COUT, CIN, 3, 3)
    bc = gates @ b  # [B, COUT]

    nc = get_program()
    res = run_bass_kernel_spmd(nc, make_in_maps(x, Wc, bc),
                               core_ids=list(range(N_CORES)))
    return (gather_y(res.results), loss)


# revision 35
# speedup vs baseline: 1.0811x; 1.0811x over previous
"""MoE gated 3x3 conv (eval path) on 8 trn2 NeuronCores.

Strategy:
- Routing (tiny: [16,64]@[64,16] -> softmax -> top-4 gates) and the scalar
  aux loss are computed on host; the gates determine a per-sample merged
  conv weight  Wc[b] = sum_e gates[b,e] * W[e]  (conv is linear in the
  weights, and zero-gate experts contribute nothing), so the device does
  exactly one 3x3 conv per sample instead of num_experts of them.
- Data parallel over batch: 2 samples per core. Sample 0 lives on SBUF
  partitions 0-63, sample 1 on partitions 64-127, so the two per-sample
  matmul chains are row-tiled (tile_position (0,0)/(64,0)) and the PE
  runs both concurrently.
- Width-65 padded image layout: row i of the padded image is
  [0, x[i-1, 0..63]], with zero rows above and below. A single zero
  column between consecutive rows serves as BOTH the right pad of row i
  and the left pad of row i+1, so every conv tap (dy, dx) is a pure flat
  shift by dy*65+dx and the matmul moving operand is fully contiguous.
- The conv is 9 shift-matmuls accumulating in PSUM over flat 512-wide
  windows (not row-aligned; junk columns are stripped on host). 9 windows
  cover the 64x65 flat output space.
- Default MODE "bf16x4": bf16 matmuls with window PAIRS on the 2x2 PE
  tile_position grid -- 4 accumulation chains (2 samples x 2 windows) run
  concurrently on the 128x128 array; fp32 PSUM accumulation; ~2.2e-3
  scale-relative output error. MODE "f32r2" is a float32r (TF32-like)
  fallback at ~1.4e-4 error, ~35% slower.
- Per-window bias-adds are split across the Vector and Scalar engines;
  x is loaded in 5 range-pieces so early windows' matmuls start while the
  rest of the image is still in flight.
"""

import ml_dtypes
import numpy as np

import concourse.bacc as bacc
import concourse.tile as tile
from concourse import mybir
from concourse.bass_utils import run_bass_kernel_spmd

N_CORES = 8
B, CIN, COUT, E = 16, 64, 64, 16
H = W_SP = 64
KTOP = 4
PW = W_SP + 1          # 65: one shared zero column per row
NROW = 73              # padded rows (top zero, data, bottom zero + overrun)
XFLAT = NROW * PW      # 4745
NMM = 416              # flat window width per PSUM accumulation group
NWIN = 10              # 10 * 416 == 64*65 exactly (no junk overrun)
XPIECES = (648, 1672, 2696, 3720, XFLAT)  # x-load split points (flat, excl.)
F32 = mybir.dt.float32

# "bf16x4": bf16 matmuls, 2 samples x 2 windows concurrent on the 2x2 PE
#           tile grid (~2.5e-3 scale-relative output error).
# "f32r2":  float32r (TF32-like) matmuls, 2 samples concurrent via row
#           tiling only (~1.4e-4 error; fp32r PSUM must start at
#           partition 0, so no column tiling).
MODE = "bf16x4"

_PROGRAM_CACHE = {}


def _routing_gates(x, w_gate):
    """Eval-path gates, mirroring the reference: softmax over clean logits,
    top-4 renormalized. [B, E] float32."""
    gate_x = x.reshape(B, CIN, H * W_SP).mean(axis=2)      # [B, Cin]
    logits = gate_x.astype(np.float32) @ w_gate            # [B, E]
    m = logits.max(axis=1, keepdims=True)
    ex = np.exp(logits - m)
    sm = ex / ex.sum(axis=1, keepdims=True)
    idx = np.argsort(-sm, axis=1, kind="stable")[:, :KTOP]
    vals = np.take_along_axis(sm, idx, axis=1)
    gk = vals / (vals.sum(axis=1, keepdims=True) + 1e-6)
    gates = np.zeros((B, E), np.float32)
    np.put_along_axis(gates, idx, gk.astype(np.float32), axis=1)
    return gates


def _aux_loss(gates):
    load = (gates > 0).sum(axis=0).astype(np.float32)
    importance = gates.sum(axis=0).astype(np.float32)

    def cv_sq(v):
        return v.var(ddof=1) / (v.mean() ** 2 + 1e-10)

    return np.float32((cv_sq(importance) + cv_sq(load)) * 0.01)


def _build_program():
    mm_dt = mybir.dt.bfloat16 if MODE == "bf16x4" else mybir.dt.float32r
    nc = bacc.Bacc("TRN2", target_bir_lowering=False, debug=False,
                   num_devices=N_CORES)
    # xw = [wct | padded x] so one DMA delivers the weights plus the first
    # windows' image data
    xw = nc.dram_tensor("xw", [128, 9 * COUT + XFLAT], mm_dt,
                        kind="ExternalInput").ap()
    bias2 = nc.dram_tensor("bias2", [128, 2], F32, kind="ExternalInput").ap()
    y2p = nc.dram_tensor("y2p", [COUT, NWIN, 2 * NMM], F32,
                         kind="ExternalOutput").ap()
    y2v = y2p  # [co, window, sample*NMM] -- window slice is contiguous 2D
    W0 = 9 * COUT

    with tile.TileContext(nc) as tc:
        with tc.tile_pool(name="xs", bufs=1) as xpool, \
             tc.tile_pool(name="w", bufs=1) as wpool, \
             tc.tile_pool(name="out", bufs=3) as opool, \
             tc.tile_pool(name="ps", bufs=3, space="PSUM") as pspool:
            xsw = xpool.tile([128, W0 + XFLAT], mm_dt)
            xs = xsw[:, W0:]
            w3 = xsw[:, 0:W0].rearrange("p (t c) -> p t c", t=9)
            bsb = wpool.tile([128, 2], F32)
            nc.sync.dma_start(out=xsw[:, 0:W0 + XPIECES[0]],
                              in_=xw[:, 0:W0 + XPIECES[0]])
            nc.scalar.dma_start(out=bsb, in_=bias2)
            lo = XPIECES[0]
            for hi in XPIECES[1:]:
                nc.sync.dma_start(out=xsw[:, W0 + lo:W0 + hi],
                                  in_=xw[:, W0 + lo:W0 + hi])
                lo = hi

            if MODE == "bf16x4":
                _body_bf16x4(nc, pspool, opool, xs, w3, bsb, y2v)
            else:
                _body_f32r2(nc, pspool, opool, xs, w3, bsb, y2v)
    nc.compile()
    return nc


def _body_f32r2(nc, pspool, opool, xs, w3, bsb, y2v):
    for g in range(NWIN):
        psA = pspool.tile([COUT, NMM], F32, tag="psA")
        psB = pspool.tile([COUT, NMM], F32, tag="psB")
        for t in range(9):
            dy, dx = divmod(t, 3)
            o = g * NMM + dy * PW + dx
            nc.tensor.matmul(psA, lhsT=w3[0:64, t, :],
                             rhs=xs[0:64, o:o + NMM],
                             start=(t == 0), stop=(t == 8))
            nc.tensor.matmul(psB, lhsT=w3[64:128, t, :],
                             rhs=xs[64:128, o:o + NMM],
                             start=(t == 0), stop=(t == 8))
        oAB = opool.tile([COUT, 2 * NMM], F32, tag="oAB")
        nc.vector.tensor_scalar_add(oAB[:, 0:NMM], psA, bsb[0:64, 0:1])
        nc.vector.tensor_scalar_add(oAB[:, NMM:], psB, bsb[0:64, 1:2])
        nc.sync.dma_start(out=y2v[:, g, :], in_=oAB)


def _body_bf16x4(nc, pspool, opool, xs, w3, bsb, y2v):
    # windows in pairs: chains (sample, window) on PE tile grid
    # (0,g)=(row0,col0) (1,g)=(row64,col0) (0,g1)=(row0,col64)
    # (1,g1)=(row64,col64); t1 holds sample-0 window g (top) and g1
    # (bottom), t2 the same for sample 1.
    for g in range(0, NWIN, 2):
        g1 = g + 1
        t1 = pspool.tile([128, NMM], F32, tag="t1")
        t2 = pspool.tile([128, NMM], F32, tag="t2")
        for t in range(9):
            dy, dx = divmod(t, 3)
            o = g * NMM + dy * PW + dx
            o1 = o + NMM
            nc.tensor.matmul(t1[0:64], lhsT=w3[0:64, t, :],
                             rhs=xs[0:64, o:o + NMM],
                             start=(t == 0), stop=(t == 8))
            nc.tensor.matmul(t2[0:64], lhsT=w3[64:128, t, :],
                             rhs=xs[64:128, o:o + NMM],
                             start=(t == 0), stop=(t == 8))
            nc.tensor.matmul(t1[64:128], lhsT=w3[0:64, t, :],
                             rhs=xs[0:64, o1:o1 + NMM],
                             start=(t == 0), stop=(t == 8))
            nc.tensor.matmul(t2[64:128], lhsT=w3[64:128, t, :],
                             rhs=xs[64:128, o1:o1 + NMM],
                             start=(t == 0), stop=(t == 8))
        oT = opool.tile([128, 2 * NMM], F32, tag="oT")
        nc.vector.tensor_scalar_add(oT[0:64, 0:NMM], t1[0:64], bsb[0:64, 0:1])
        nc.scalar.add(oT[0:64, NMM:], t2[0:64], bsb[0:64, 1:2])
        nc.sync.dma_start(out=y2v[:, g, :], in_=oT[0:64])
        nc.vector.tensor_scalar_add(oT[64:128, 0:NMM], t1[64:128],
                                    bsb[64:128, 0:1])
        nc.scalar.add(oT[64:128, NMM:], t2[64:128], bsb[64:128, 1:2])
        nc.sync.dma_start(out=y2v[:, g1, :], in_=oT[64:128])


def get_program():
    if "nc" not in _PROGRAM_CACHE:
        _PROGRAM_CACHE["nc"] = _build_program()
    return _PROGRAM_CACHE["nc"]


def _pad_x(xpair):
    """[2, CIN, H, W] -> [128, XFLAT] width-65 padded flat layout."""
    out = np.zeros((2, CIN, NROW, PW), np.float32)
    out[:, :, 1:H + 1, 1:] = xpair
    return out.reshape(2 * CIN, XFLAT)


def make_in_maps(x, Wc, bc):
    """Per-core input maps: 2 samples per core."""
    in_maps = []
    for c in range(N_CORES):
        s0, s1 = 2 * c, 2 * c + 1
        # wct[ci + 64*s, t*64 + co] = Wc[sample, co, ci, t]
        w0 = Wc[s0].reshape(COUT, CIN, 9).transpose(1, 2, 0)
        w1 = Wc[s1].reshape(COUT, CIN, 9).transpose(1, 2, 0)
        wctm = np.concatenate([w0, w1], axis=0).reshape(128, 9 * COUT)
        bias2 = np.stack([bc[s0], bc[s1]], axis=1)      # [COUT, 2]
        bias2 = np.concatenate([bias2, bias2], axis=0)  # both psum halves
        xwm = np.concatenate([wctm, _pad_x(x[s0:s1 + 1])], axis=1)
        if MODE == "bf16x4":
            xwm = xwm.astype(ml_dtypes.bfloat16)
        in_maps.append({
            "xw": xwm,
            "bias2": np.ascontiguousarray(bias2),
        })
    return in_maps


# compaction index: y[.., h, w] = y2p[.., GIDX[h, w], JIDX[h, w]]
_f = np.arange(H)[:, None] * PW + np.arange(W_SP)[None, :]
GIDX = _f // NMM
JIDX = _f % NMM


def gather_y(results):
    y = np.empty((B, COUT, H, W_SP), np.float32)
    for c in range(N_CORES):
        yp = results[c]["y2p"].reshape(COUT, NWIN, 2, NMM)
        for s in range(2):
            y[2 * c + s] = yp[:, GIDX, s, JIDX]
    return y


def kernel(**inputs):
    x = np.asarray(inputs["x"], dtype=np.float32)
    w_gate = np.asarray(inputs["w_gate"], dtype=np.float32)
    W = np.asarray(inputs["W"], dtype=np.float32)
    b = np.asarray(inputs["b"], dtype=np.float32)
    # train is eval-only in the reference; the noise branch never runs.

    gates = _routing_gates(x, w_gate)
    loss = _aux_loss(gates)
    Wc = np.tensordot(gates, W.reshape(E, -1), axes=(1, 0)) \
        .reshape(B, COUT, CIN, 3, 3)
    bc = gates @ b  # [B, COUT]

    nc = get_program()
    res = run_bass_kernel_spmd(nc, make_in_maps(x, Wc, bc),
                               core_ids=list(range(N_CORES)))
    return (gather_y(res.results), loss)


# revision 36
# speedup vs baseline: 1.0840x; 1.0027x over previous
"""MoE gated 3x3 conv (eval path) on 8 trn2 NeuronCores.

Strategy:
- Routing (tiny: [16,64]@[64,16] -> softmax -> top-4 gates) and the scalar
  aux loss are computed on host; the gates determine a per-sample merged
  conv weight  Wc[b] = sum_e gates[b,e] * W[e]  (conv is linear in the
  weights, and zero-gate experts contribute nothing), so the device does
  exactly one 3x3 conv per sample instead of num_experts of them.
- Data parallel over batch: 2 samples per core. Sample 0 lives on SBUF
  partitions 0-63, sample 1 on partitions 64-127, so the two per-sample
  matmul chains are row-tiled (tile_position (0,0)/(64,0)) and the PE
  runs both concurrently.
- Width-65 padded image layout: row i of the padded image is
  [0, x[i-1, 0..63]], with zero rows above and below. A single zero
  column between consecutive rows serves as BOTH the right pad of row i
  and the left pad of row i+1, so every conv tap (dy, dx) is a pure flat
  shift by dy*65+dx and the matmul moving operand is fully contiguous.
- The conv is 9 shift-matmuls accumulating in PSUM over flat 512-wide
  windows (not row-aligned; junk columns are stripped on host). 9 windows
  cover the 64x65 flat output space.
- Default MODE "bf16x4": bf16 matmuls with window PAIRS on the 2x2 PE
  tile_position grid -- 4 accumulation chains (2 samples x 2 windows) run
  concurrently on the 128x128 array; fp32 PSUM accumulation; ~2.2e-3
  scale-relative output error. MODE "f32r2" is a float32r (TF32-like)
  fallback at ~1.4e-4 error, ~35% slower.
- Per-window bias-adds are split across the Vector and Scalar engines;
  x is loaded in 5 range-pieces so early windows' matmuls start while the
  rest of the image is still in flight.
"""

import ml_dtypes
import numpy as np

import concourse.bacc as bacc
import concourse.tile as tile
from concourse import mybir
from concourse.bass_utils import run_bass_kernel_spmd

N_CORES = 8
B, CIN, COUT, E = 16, 64, 64, 16
H = W_SP = 64
KTOP = 4
PW = W_SP + 1          # 65: one shared zero column per row
NROW = 73              # padded rows (top zero, data, bottom zero + overrun)
XFLAT = NROW * PW      # 4745
NMM = 416              # flat window width per PSUM accumulation group
NWIN = 10              # 10 * 416 == 64*65 exactly (no junk overrun)
XPIECES = (968, 1800, 2632, 3464, XFLAT)  # x-load split points (flat, excl.)
F32 = mybir.dt.float32

# "bf16x4": bf16 matmuls, 2 samples x 2 windows concurrent on the 2x2 PE
#           tile grid (~2.5e-3 scale-relative output error).
# "f32r2":  float32r (TF32-like) matmuls, 2 samples concurrent via row
#           tiling only (~1.4e-4 error; fp32r PSUM must start at
#           partition 0, so no column tiling).
MODE = "bf16x4"

_PROGRAM_CACHE = {}


def _routing_gates(x, w_gate):
    """Eval-path gates, mirroring the reference: softmax over clean logits,
    top-4 renormalized. [B, E] float32."""
    gate_x = x.reshape(B, CIN, H * W_SP).mean(axis=2)      # [B, Cin]
    logits = gate_x.astype(np.float32) @ w_gate            # [B, E]
    m = logits.max(axis=1, keepdims=True)
    ex = np.exp(logits - m)
    sm = ex / ex.sum(axis=1, keepdims=True)
    idx = np.argsort(-sm, axis=1, kind="stable")[:, :KTOP]
    vals = np.take_along_axis(sm, idx, axis=1)
    gk = vals / (vals.sum(axis=1, keepdims=True) + 1e-6)
    gates = np.zeros((B, E), np.float32)
    np.put_along_axis(gates, idx, gk.astype(np.float32), axis=1)
    return gates


def _aux_loss(gates):
    load = (gates > 0).sum(axis=0).astype(np.float32)
    importance = gates.sum(axis=0).astype(np.float32)

    def cv_sq(v):
        return v.var(ddof=1) / (v.mean() ** 2 + 1e-10)

    return np.float32((cv_sq(importance) + cv_sq(load)) * 0.01)


def _build_program():
    mm_dt = mybir.dt.bfloat16 if MODE == "bf16x4" else mybir.dt.float32r
    nc = bacc.Bacc("TRN2", target_bir_lowering=False, debug=False,
                   num_devices=N_CORES)
    # xw = [wct | padded x] so one DMA delivers the weights plus the first
    # windows' image data
    xw = nc.dram_tensor("xw", [128, 9 * COUT + XFLAT], mm_dt,
                        kind="ExternalInput").ap()
    bias2 = nc.dram_tensor("bias2", [128, 2], F32, kind="ExternalInput").ap()
    y2p = nc.dram_tensor("y2p", [COUT, NWIN, 2 * NMM], F32,
                         kind="ExternalOutput").ap()
    y2v = y2p  # [co, window, sample*NMM] -- window slice is contiguous 2D
    W0 = 9 * COUT

    with tile.TileContext(nc) as tc:
        with tc.tile_pool(name="xs", bufs=1) as xpool, \
             tc.tile_pool(name="w", bufs=1) as wpool, \
             tc.tile_pool(name="out", bufs=3) as opool, \
             tc.tile_pool(name="ps", bufs=3, space="PSUM") as pspool:
            xsw = xpool.tile([128, W0 + XFLAT], mm_dt)
            xs = xsw[:, W0:]
            w3 = xsw[:, 0:W0].rearrange("p (t c) -> p t c", t=9)
            bsb = wpool.tile([128, 2], F32)
            nc.sync.dma_start(out=xsw[:, 0:W0 + XPIECES[0]],
                              in_=xw[:, 0:W0 + XPIECES[0]])
            nc.scalar.dma_start(out=bsb, in_=bias2)
            lo = XPIECES[0]
            for hi in XPIECES[1:]:
                nc.sync.dma_start(out=xsw[:, W0 + lo:W0 + hi],
                                  in_=xw[:, W0 + lo:W0 + hi])
                lo = hi

            if MODE == "bf16x4":
                _body_bf16x4(nc, pspool, opool, xs, w3, bsb, y2v)
            else:
                _body_f32r2(nc, pspool, opool, xs, w3, bsb, y2v)
    nc.compile()
    return nc


def _body_f32r2(nc, pspool, opool, xs, w3, bsb, y2v):
    for g in range(NWIN):
        psA = pspool.tile([COUT, NMM], F32, tag="psA")
        psB = pspool.tile([COUT, NMM], F32, tag="psB")
        for t in range(9):
            dy, dx = divmod(t, 3)
            o = g * NMM + dy * PW + dx
            nc.tensor.matmul(psA, lhsT=w3[0:64, t, :],
                             rhs=xs[0:64, o:o + NMM],
                             start=(t == 0), stop=(t == 8))
            nc.tensor.matmul(psB, lhsT=w3[64:128, t, :],
                             rhs=xs[64:128, o:o + NMM],
                             start=(t == 0), stop=(t == 8))
        oAB = opool.tile([COUT, 2 * NMM], F32, tag="oAB")
        nc.vector.tensor_scalar_add(oAB[:, 0:NMM], psA, bsb[0:64, 0:1])
        nc.vector.tensor_scalar_add(oAB[:, NMM:], psB, bsb[0:64, 1:2])
        nc.sync.dma_start(out=y2v[:, g, :], in_=oAB)


def _body_bf16x4(nc, pspool, opool, xs, w3, bsb, y2v):
    # windows in pairs: chains (sample, window) on PE tile grid
    # (0,g)=(row0,col0) (1,g)=(row64,col0) (0,g1)=(row0,col64)
    # (1,g1)=(row64,col64); t1 holds sample-0 window g (top) and g1
    # (bottom), t2 the same for sample 1.
    for g in range(0, NWIN, 2):
        g1 = g + 1
        t1 = pspool.tile([128, NMM], F32, tag="t1")
        t2 = pspool.tile([128, NMM], F32, tag="t2")
        for t in range(9):
            dy, dx = divmod(t, 3)
            o = g * NMM + dy * PW + dx
            o1 = o + NMM
            nc.tensor.matmul(t1[0:64], lhsT=w3[0:64, t, :],
                             rhs=xs[0:64, o:o + NMM],
                             start=(t == 0), stop=(t == 8))
            nc.tensor.matmul(t2[0:64], lhsT=w3[64:128, t, :],
                             rhs=xs[64:128, o:o + NMM],
                             start=(t == 0), stop=(t == 8))
            nc.tensor.matmul(t1[64:128], lhsT=w3[0:64, t, :],
                             rhs=xs[0:64, o1:o1 + NMM],
                             start=(t == 0), stop=(t == 8))
            nc.tensor.matmul(t2[64:128], lhsT=w3[64:128, t, :],
                             rhs=xs[64:128, o1:o1 + NMM],
                             start=(t == 0), stop=(t == 8))
        oT = opool.tile([128, 2 * NMM], F32, tag="oT")
        nc.vector.tensor_scalar_add(oT[0:64, 0:NMM], t1[0:64], bsb[0:64, 0:1])
        nc.scalar.add(oT[0:64, NMM:], t2[0:64], bsb[0:64, 1:2])
        nc.sync.dma_start(out=y2v[:, g, :], in_=oT[0:64])
        nc.vector.tensor_scalar_add(oT[64:128, 0:NMM], t1[64:128],
                                    bsb[64:128, 0:1])
        nc.scalar.add(oT[64:128, NMM:], t2[64:128], bsb[64:128, 1:2])
        nc.sync.dma_start(out=y2v[:, g1, :], in_=oT[64:128])


def get_program():
    if "nc" not in _PROGRAM_CACHE:
        _PROGRAM_CACHE["nc"] = _build_program()
    return _PROGRAM_CACHE["nc"]


def _pad_x(xpair):
    """[2, CIN, H, W] -> [128, XFLAT] width-65 padded flat layout."""
    out = np.zeros((2, CIN, NROW, PW), np.float32)
    out[:, :, 1:H + 1, 1:] = xpair
    return out.reshape(2 * CIN, XFLAT)


def make_in_maps(x, Wc, bc):
    """Per-core input maps: 2 samples per core."""
    in_maps = []
    for c in range(N_CORES):
        s0, s1 = 2 * c, 2 * c + 1
        # wct[ci + 64*s, t*64 + co] = Wc[sample, co, ci, t]
        w0 = Wc[s0].reshape(COUT, CIN, 9).transpose(1, 2, 0)
        w1 = Wc[s1].reshape(COUT, CIN, 9).transpose(1, 2, 0)
        wctm = np.concatenate([w0, w1], axis=0).reshape(128, 9 * COUT)
        bias2 = np.stack([bc[s0], bc[s1]], axis=1)      # [COUT, 2]
        bias2 = np.concatenate([bias2, bias2], axis=0)  # both psum halves
        xwm = np.concatenate([wctm, _pad_x(x[s0:s1 + 1])], axis=1)
        if MODE == "bf16x4":
            xwm = xwm.astype(ml_dtypes.bfloat16)
        in_maps.append({
            "xw": xwm,
            "bias2": np.ascontiguousarray(bias2),
        })
    return in_maps


# compaction index: y[.., h, w] = y2p[.., GIDX[h, w], JIDX[h, w]]
_f = np.arange(H)[:, None] * PW + np.arange(W_SP)[None, :]
GIDX = _f // NMM
JIDX = _f % NMM


def gather_y(results):
    y = np.empty((B, COUT, H, W_SP), np.float32)
    for c in range(N_CORES):
        yp = results[c]["y2p"].reshape(COUT, NWIN, 2, NMM)
        for s in range(2):
            y[2 * c + s] = yp[:, GIDX, s, JIDX]
    return y


def kernel(**inputs):
    x = np.asarray(inputs["x"], dtype=np.float32)
    w_gate = np.asarray(inputs["w_gate"], dtype=np.float32)
    W = np.asarray(inputs["W"], dtype=np.float32)
    b = np.asarray(inputs["b"], dtype=np.float32)
    # train is eval-only in the reference; the noise branch never runs.

    gates = _routing_gates(x, w_gate)
    loss = _aux_loss(gates)
    Wc = np.tensordot(gates, W.reshape(E, -1), axes=(1, 0)) \
        .reshape(B, COUT, CIN, 3, 3)
    bc = gates @ b  # [B, COUT]

    nc = get_program()
    res = run_bass_kernel_spmd(nc, make_in_maps(x, Wc, bc),
                               core_ids=list(range(N_CORES)))
    return (gather_y(res.results), loss)
